# revision 4
# baseline (speedup 1.0000x reference)
"""GAT (2-layer, 8-head) Trainium2 Bass kernel, 8-core SPMD.

Strategy (dst-sharded edge partition):
- Host: append self-loops, shard edges by dst range (6250 dsts/core), bucket
  into 50 windows of 125 dsts, split each window's edges by src<32768 (lo/hi
  for int16 dma_gather indexing), pad sections to 128-edge chunks with
  SPMD-uniform (max-over-cores) static sizes.
- Device phase 1: sharded matmul x_sliceT @ W1ext -> h rows
  [h(256) | a_src(8) | a_dst(8) | pad] bf16, AllGather -> full 50000-row table.
- Device phase 2 (per window): dma_gather h[src] rows (768B); e =
  lrelu(a_src[src] + a_dst[dst]) with a_dst broadcast via one-hot S^T matmul;
  w = exp(e); segment-sum s and alpha-weighted aggregation via one-hot S
  matmuls accumulated in PSUM; out = (S^T w h)/s; +b1, ELU -> act1 (stored
  transposed for the next matmul).
- Phase 3: act1T @ W2ext -> h2 rows [h2(10)|a_s2|a_d2|pad] bf16, AllGather.
- Phase 4: same edge pipeline with 1 head, 10 channels -> final [6250, 10]
  fp32 slice per core; host concatenates.
"""
import os
import sys
from contextlib import ExitStack

for _p in ("/opt/trn_rl_repo", os.path.expanduser("~/.axon_site/_ro/trn_rl_repo")):
    if os.path.isdir(_p) and _p not in sys.path:
        sys.path.insert(0, _p)

import numpy as np
import ml_dtypes

P = 128


class Cfg:
    def __init__(self, N=50000, F=767, HEADS=8, CH=32, NCLS=10, NCORES=8,
                 WD=125, NW=50, SPLIT=32768, G=2, NEG=0.2):
        self.N, self.F, self.HEADS, self.CH, self.NCLS = N, F, HEADS, CH, NCLS
        self.NCORES, self.WD, self.NW, self.SPLIT, self.G, self.NEG = (
            NCORES, WD, NW, SPLIT, G, NEG)
        self.HID = HEADS * CH                      # 256
        self.DPC = WD * NW                         # dsts per core
        assert self.DPC * NCORES == N
        self.FP = (F + P - 1) // P * P             # padded F
        self.KC1 = self.FP // P                    # k-chunks layer 1
        self.RT = (self.DPC + P - 1) // P          # row tiles per core
        self.DPCP = self.RT * P                    # padded rows/core
        self.W1C = self.HID + 2 * HEADS            # 272 used cols
        self.T1 = 384                              # padded L1 table width (768B rows)
        assert self.T1 * 2 % 256 == 0 and self.W1C <= self.T1
        self.KC2 = self.HID // P                   # 2
        self.W2C = NCLS + 2                        # 12 used cols
        self.T2 = 128                              # padded L2 table width (256B rows)


def _wrap_idxs(idx_list):
    """int16 idx list -> [128, ceil(n/16)] wrapped (p=j%16, col=j//16), x8."""
    n = len(idx_list)
    cols = max(1, (n + 15) // 16)
    arr = np.zeros((16, cols), dtype=np.int16)
    if n:
        j = np.arange(n)
        arr[j % 16, j // 16] = idx_list
    return np.tile(arr, (8, 1))


def preprocess(cfg, x, edge_index, W1, att_src1, att_dst1, b1, W2, att_src2,
               att_dst2, b2):
    c = cfg
    N, E = c.N, edge_index.shape[1]
    src = np.concatenate([edge_index[0], np.arange(N)]).astype(np.int64)
    dst = np.concatenate([edge_index[1], np.arange(N)]).astype(np.int64)

    # --- weight prep (param folding only) ---
    W1 = np.asarray(W1, np.float32)
    a_s1 = np.asarray(att_src1, np.float32)
    a_d1 = np.asarray(att_dst1, np.float32)
    W1e = np.zeros((c.FP, c.T1), np.float32)
    W1e[: c.F, : c.HID] = W1
    for h in range(c.HEADS):
        blk = W1[:, h * c.CH : (h + 1) * c.CH]
        W1e[: c.F, c.HID + h] = blk @ a_s1[h]
        W1e[: c.F, c.HID + c.HEADS + h] = blk @ a_d1[h]
    W2 = np.asarray(W2, np.float32)
    W2e = np.zeros((c.HID, c.T2), np.float32)
    W2e[:, : c.NCLS] = W2
    W2e[:, c.NCLS] = W2 @ np.asarray(att_src2, np.float32)[0]
    W2e[:, c.NCLS + 1] = W2 @ np.asarray(att_dst2, np.float32)[0]

    # --- per-core edge bucketing ---
    core = dst // c.DPC
    dloc = dst - core * c.DPC
    win = dloc // c.WD
    dcol = dloc % c.WD
    lists = [[([], []) for _ in range(c.NW)] for _ in range(c.NCORES)]
    is_lo = src < c.SPLIT
    order = np.lexsort((win, core))
    for i in order:
        co, w = int(core[i]), int(win[i])
        lists[co][w][0 if is_lo[i] else 1].append((int(src[i]), int(dcol[i])))

    def nchunks(n):
        return (n + P - 1) // P

    LC = [max(nchunks(len(lists[co][w][0])) for co in range(c.NCORES))
          for w in range(c.NW)]
    HC = [max(nchunks(len(lists[co][w][1])) for co in range(c.NCORES))
          for w in range(c.NW)]

    # group layout: for each group g of G windows: lo sections then hi sections
    NG = (c.NW + c.G - 1) // c.G
    meta = {"LC": LC, "HC": HC, "NG": NG, "groups": []}
    # chunk map: list over groups of dict(lo_chunks={w: (start,count)},...)
    chunk_cursor = 0
    idx_cols = 0
    for g in range(NG):
        ws = list(range(g * c.G, min((g + 1) * c.G, c.NW)))
        lo_n = sum(LC[w] for w in ws)
        hi_n = sum(HC[w] for w in ws)
        ginfo = {"ws": ws, "lo_n": lo_n, "hi_n": hi_n,
                 "chunk0": chunk_cursor, "idx_col0": idx_cols,
                 "lo": {}, "hi": {}}
        off = chunk_cursor
        for w in ws:
            ginfo["lo"][w] = (off, LC[w]); off += LC[w]
        for w in ws:
            ginfo["hi"][w] = (off, HC[w]); off += HC[w]
        chunk_cursor = off
        idx_cols += (lo_n + hi_n) * (P // 16)
        meta["groups"].append(ginfo)
    TOTC = chunk_cursor
    meta["TOTC"] = TOTC
    meta["idx_cols"] = idx_cols

    # --- per-core arrays ---
    in_maps = []
    xf = np.asarray(x, np.float32)
    for co in range(c.NCORES):
        idx_parts = []
        dstc = np.full((P, TOTC), 126.0, np.float32)
        for g in meta["groups"]:
            for kind, base_key in (("lo", "lo"), ("hi", "hi")):
                sec = []
                for w in g["ws"]:
                    pairs = lists[co][w][0 if kind == "lo" else 1]
                    cstart, ccount = g[base_key][w]
                    nslots = ccount * P
                    vals = np.zeros(nslots, np.int16)
                    for j, (s, dc) in enumerate(pairs):
                        vals[j] = s if kind == "lo" else s - c.SPLIT
                        dstc[j % P, cstart + j // P] = float(dc)
                    # pad slots: idx 0 (valid row), dstcol stays 126 (discard)
                    sec.append(vals)
                sec = np.concatenate(sec) if sec else np.zeros(0, np.int16)
                idx_parts.append(_wrap_idxs(sec) if len(sec) else
                                 np.zeros((P, 1), np.int16)[:, :0])
        idx_np = (np.concatenate(idx_parts, axis=1) if idx_parts
                  else np.zeros((P, 0), np.int16))
        assert idx_np.shape[1] == meta["idx_cols"], (idx_np.shape, meta["idx_cols"])

        xT = np.zeros((c.FP, c.DPCP), ml_dtypes.bfloat16)
        xs = xf[co * c.DPC : (co + 1) * c.DPC]
        xT[: c.F, : c.DPC] = xs.T.astype(ml_dtypes.bfloat16)

        in_maps.append({
            "xT": xT,
            "W1e": W1e.astype(ml_dtypes.bfloat16),
            "W2e": W2e.astype(ml_dtypes.bfloat16),
            "idx": idx_np,
            "dstc": dstc,
            "iota_row": np.tile(np.arange(P, dtype=np.float32), (P, 1)),
            "iota_col": np.arange(P, dtype=np.float32).reshape(P, 1),
            "b1r": np.tile(np.asarray(b1, np.float32)[None, :], (P, 1)),
            "b2r": np.tile(np.asarray(b2, np.float32)[None, :], (P, 1)),
        })
    return meta, in_maps


def build_program(cfg, meta):
    import concourse.bacc as bacc
    import concourse.bass as bass
    import concourse.mybir as mybir
    import concourse.tile as tile
    from concourse.library_config import mlp
    from concourse.masks import make_identity

    c = cfg
    f32, bf16 = mybir.dt.float32, mybir.dt.bfloat16
    AT = mybir.ActivationFunctionType
    OP = mybir.AluOpType

    nc = bacc.Bacc("TRN2", target_bir_lowering=False, debug=False,
                   num_devices=c.NCORES, num_swdge_queues=4)
    xT_d = nc.dram_tensor("xT", [c.FP, c.DPCP], bf16, kind="ExternalInput")
    W1e_d = nc.dram_tensor("W1e", [c.FP, c.T1], bf16, kind="ExternalInput")
    W2e_d = nc.dram_tensor("W2e", [c.HID, c.T2], bf16, kind="ExternalInput")
    idx_d = nc.dram_tensor("idx", [P, max(1, meta["idx_cols"])], mybir.dt.int16,
                           kind="ExternalInput")
    dstc_d = nc.dram_tensor("dstc", [P, meta["TOTC"]], f32, kind="ExternalInput")
    iro_d = nc.dram_tensor("iota_row", [P, P], f32, kind="ExternalInput")
    ico_d = nc.dram_tensor("iota_col", [P, 1], f32, kind="ExternalInput")
    b1r_d = nc.dram_tensor("b1r", [P, c.HID], f32, kind="ExternalInput")
    b2r_d = nc.dram_tensor("b2r", [P, c.NCLS], f32, kind="ExternalInput")
    out_d = nc.dram_tensor("out", [c.DPC, c.NCLS], f32, kind="ExternalOutput")

    # internal DRAM
    h_bounce = nc.dram_tensor("h_bounce", [c.DPCP, c.T1], bf16, kind="Internal")
    _shared = "Shared" if c.NCORES > 4 else "Local"
    h_tab = nc.dram_tensor("h_tab", [c.N, c.T1], bf16, kind="Internal",
                           addr_space=_shared)
    ad1_sl = nc.dram_tensor("ad1_sl", [c.DPCP, c.HEADS], bf16, kind="Internal")
    a1T_dram = nc.dram_tensor("a1T", [c.HID, c.DPCP], bf16, kind="Internal")
    h2_bounce = nc.dram_tensor("h2_bounce", [c.DPCP, c.T2], bf16, kind="Internal")
    h2_tab = nc.dram_tensor("h2_tab", [c.N, c.T2], bf16, kind="Internal",
                            addr_space=_shared)
    ad2_sl = nc.dram_tensor("ad2_sl", [c.DPCP, 1], bf16, kind="Internal")

    groups = meta["groups"]
    LC, HC = meta["LC"], meta["HC"]

    with ExitStack() as stack:
        tc = stack.enter_context(tile.TileContext(nc))
        cpool = stack.enter_context(tc.tile_pool(name="consts", bufs=1))
        nc.gpsimd.load_library(mlp)

        ident = cpool.tile([P, P], f32)
        make_identity(nc, ident[:])
        iro_t = cpool.tile([P, P], f32)
        nc.sync.dma_start(iro_t[:], iro_d[:])
        ico_t = cpool.tile([P, 1], f32)
        nc.sync.dma_start(ico_t[:], ico_d[:])
        b1r_t = cpool.tile([P, c.HID], f32)
        nc.sync.dma_start(b1r_t[:], b1r_d[:])
        b2r_t = cpool.tile([P, c.NCLS], f32)
        nc.sync.dma_start(b2r_t[:], b2r_d[:])

        # ---------------- phase 1: L1 matmul (sharded rows) ----------------
        with tc.tile_pool(name="mm1", bufs=1) as mm1, \
             tc.tile_pool(name="mm1w", bufs=2) as mm1w, \
             tc.tile_pool(name="mm1p", bufs=2, space="PSUM") as mm1p:
            xts = []
            for k in range(c.KC1):
                t = mm1.tile([P, c.DPCP], bf16, tag=f"xts{k}")
                nc.sync.dma_start(t[:], xT_d[k * P : (k + 1) * P, :])
                xts.append(t)
            w1s = []
            for k in range(c.KC1):
                t = mm1.tile([P, c.T1], bf16, tag=f"w1s{k}")
                nc.sync.dma_start(t[:], W1e_d[k * P : (k + 1) * P, :])
                w1s.append(t)
            for r in range(c.RT):
                ps = mm1p.tile([P, c.T1], f32, space="PSUM", tag="mmps")
                for k in range(c.KC1):
                    nc.tensor.matmul(
                        ps[:], lhsT=xts[k][:, r * P : (r + 1) * P],
                        rhs=w1s[k][:], start=(k == 0), stop=(k == c.KC1 - 1))
                hsb = mm1w.tile([P, c.T1], bf16, tag="hsb")
                nc.scalar.copy(hsb[:], ps[:])
                nc.sync.dma_start(h_bounce[r * P : (r + 1) * P, :], hsb[:])
                nc.sync.dma_start(
                    ad1_sl[r * P : (r + 1) * P, :],
                    hsb[:, c.HID + c.HEADS : c.HID + 2 * c.HEADS])

        nc.gpsimd.collective_compute(
            "AllGather", OP.bypass,
            replica_groups=[list(range(c.NCORES))],
            ins=[h_bounce.ap()[0 : c.DPC, :]],
            outs=[h_tab.ap()])

        # ---------------- phase 2: L1 edge windows ----------------
        NCH1 = max(LC[w] + HC[w] for w in range(c.NW))
        with tc.tile_pool(name="eg", bufs=2) as eg, \
             tc.tile_pool(name="emeta", bufs=1) as emeta, \
             tc.tile_pool(name="ew", bufs=2) as ew, \
             tc.tile_pool(name="eS", bufs=NCH1 + 4) as eSp, \
             tc.tile_pool(name="est", bufs=3) as estp, \
             tc.tile_pool(name="ep1", bufs=2, space="PSUM") as ep1, \
             tc.tile_pool(name="ep2", bufs=2, space="PSUM") as ep2, \
             tc.tile_pool(name="ep3", bufs=2, space="PSUM") as ep3:
            idx_t = emeta.tile([P, max(1, meta["idx_cols"])], mybir.dt.int16)
            nc.sync.dma_start(idx_t[:], idx_d[:])
            dstc_t = emeta.tile([P, meta["TOTC"]], f32)
            nc.sync.dma_start(dstc_t[:], dstc_d[:])

            def edge_phase(tab_d, tabw, ad_sl_d, adw, heads, ch, gelem,
                           epilogue):
                """Shared window pipeline for both layers.
                tabw: table width; adw: #a_dst cols; gelem: gathered row elems;
                epilogue(w, out_ps, s_ps) -> emits the per-window tail."""
                hc = heads * ch
                for g in groups:
                    GC = g["lo_n"] + g["hi_n"]
                    gt = eg.tile([P, GC, gelem], bf16, tag="gt")
                    col0 = g["idx_col0"]
                    qn = [0]
                    def gather(sec_n, col_off, out_off, in_ap):
                        if sec_n == 0:
                            return
                        nidx = sec_n * P
                        nc.gpsimd.dma_gather(
                            gt[:, out_off : out_off + sec_n, :], in_ap,
                            idx_t[:, col_off : col_off + nidx // 16],
                            nidx, nidx, gelem, single_packet=False,
                            queue_num=qn[0] % 4)
                        qn[0] += 1
                    gather(g["lo_n"], col0, 0, tab_d.ap())
                    gather(g["hi_n"], col0 + g["lo_n"] * (P // 16), g["lo_n"],
                           tab_d.ap()[c.SPLIT :, :])

                    for w in g["ws"]:
                        ncw = LC[w] + HC[w] if tabw == c.T1 else LC[w] + HC[w]
                        spans = [g["lo"][w], g["hi"][w]]
                        spans = [(s - g["chunk0"], n) for (s, n) in spans if n]
                        nch = sum(n for _, n in spans)
                        if nch == 0:
                            continue
                        # a_dst window rows
                        ad_t = ew.tile([P, adw], bf16, tag="ad")
                        nc.vector.memset(ad_t[:], 0.0)
                        nc.sync.dma_start(
                            ad_t[0 : c.WD, :],
                            ad_sl_d[w * c.WD : (w + 1) * c.WD, :])
                        # pass A: S, S^T, ed per chunk
                        S_tiles = []
                        ed_ps = ep1.tile([P, nch, adw], f32, space="PSUM",
                                         tag="edps")
                        ci = 0
                        for s0, n in spans:
                            for k in range(n):
                                gci = g["chunk0"] + s0 + k   # global chunk
                                lci = s0 + k                 # chunk in gt
                                dcol = dstc_t[:, gci : gci + 1]
                                S = eSp.tile([P, P], bf16, tag="S")
                                nc.vector.tensor_scalar(
                                    out=S[:], in0=iro_t[:], scalar1=dcol,
                                    scalar2=None, op0=OP.is_equal)
                                tp = ep3.tile([P, P], f32, space="PSUM",
                                              tag="tp")
                                nc.tensor.transpose(
                                    out=tp[:], in_=dcol.to_broadcast([P, P]),
                                    identity=ident[:])
                                ST = estp.tile([P, P], bf16, tag="ST")
                                nc.vector.tensor_tensor(
                                    out=ST[:], in0=tp[:],
                                    in1=ico_t[:].to_broadcast([P, P]),
                                    op=OP.is_equal)
                                nc.tensor.matmul(
                                    ed_ps[:, ci, :], lhsT=ST[:], rhs=ad_t[:],
                                    start=True, stop=True)
                                S_tiles.append((S, lci))
                                ci += 1
                        # e chain (batched over window chunks)
                        e_t = ew.tile([P, nch, adw], f32, tag="e")
                        es_parts_done = False
                        # es = gathered a_src cols; contiguity: per span slice
                        # (gt chunk ranges may be non-adjacent across spans)
                        ci = 0
                        for s0, n in spans:
                            nc.vector.tensor_tensor(
                                out=e_t[:, ci : ci + n, :],
                                in0=gt[:, s0 : s0 + n, hc : hc + adw
                                       ] if heads > 1 or True else None,
                                in1=ed_ps[:, ci : ci + n, :], op=OP.add)
                            ci += n
                        lr_t = ew.tile([P, nch, adw], f32, tag="lr")
                        nc.vector.tensor_scalar_mul(lr_t[:], e_t[:], c.NEG)
                        nc.vector.tensor_tensor(
                            out=lr_t[:], in0=lr_t[:], in1=e_t[:], op=OP.max)
                        w_t = ew.tile([P, nch, adw], bf16, tag="w")
                        nc.scalar.activation(w_t[:], lr_t[:], AT.Exp)
                        # pass B: segment sums + weighted aggregation
                        s_ps = ep2.tile([P, adw], f32, space="PSUM", tag="sps")
                        for ci, (S, lci) in enumerate(S_tiles):
                            nc.tensor.matmul(
                                s_ps[:], lhsT=S[:], rhs=w_t[:, ci, :],
                                start=(ci == 0), stop=(ci == len(S_tiles) - 1))
                        # msg: in-place h *= w (broadcast over ch)
                        ci = 0
                        for s0, n in spans:
                            nc.vector.tensor_tensor(
                                out=gt[:, s0 : s0 + n, 0 : hc].rearrange(
                                    "p c (h x) -> p c h x", h=heads),
                                in0=gt[:, s0 : s0 + n, 0 : hc].rearrange(
                                    "p c (h x) -> p c h x", h=heads),
                                in1=w_t[:, ci : ci + n, :].to_broadcast(
                                    [P, n, adw, ch]),
                                op=OP.mult)
                            ci += n
                        out_ps = ep2.tile([P, hc], f32, space="PSUM", tag="ops")
                        k = 0
                        for s0, n in spans:
                            for j in range(n):
                                S, lci = S_tiles[k]; k += 1
                                nc.tensor.matmul(
                                    out_ps[:], lhsT=S[:],
                                    rhs=gt[:, lci, 0 : hc],
                                    start=(k == 1), stop=(k == len(S_tiles)))
                        epilogue(w, out_ps, s_ps)

            # ---- L1 epilogue ----
            def epi1(w, out_ps, s_ps):
                s_sb = ew.tile([P, c.HEADS], f32, tag="ssb")
                nc.vector.tensor_scalar_add(s_sb[:], s_ps[:], 1e-16)
                rs = ew.tile([P, c.HEADS], f32, tag="rs")
                nc.vector.reciprocal(rs[:], s_sb[:])
                z = ew.tile([P, c.HID], f32, tag="z")
                nc.vector.tensor_tensor(
                    out=z[:].rearrange("p (h x) -> p h x", h=c.HEADS),
                    in0=out_ps[:].rearrange("p (h x) -> p h x", h=c.HEADS),
                    in1=rs[:].to_broadcast([P, c.HEADS, c.CH]), op=OP.mult)
                nc.vector.tensor_add(out=z[:], in0=z[:], in1=b1r_t[:])
                mn = ew.tile([P, c.HID], f32, tag="mn")
                nc.vector.tensor_scalar_min(mn[:], z[:], 0.0)
                em = ew.tile([P, c.HID], f32, tag="em")
                nc.scalar.activation(em[:], mn[:], AT.Exp)
                nc.vector.tensor_scalar_max(mn[:], z[:], 0.0)
                nc.vector.tensor_add(out=em[:], in0=em[:], in1=mn[:])
                nc.vector.tensor_scalar_add(em[:], em[:], -1.0)
                for half in range(c.HID // P):
                    tp = ep3.tile([P, P], f32, space="PSUM", tag="tp")
                    nc.tensor.transpose(
                        out=tp[:], in_=em[:, half * P : (half + 1) * P],
                        identity=ident[:])
                    a1c = ew.tile([P, P], bf16, tag="a1c")
                    nc.scalar.copy(a1c[:], tp[:])
                    nc.sync.dma_start(
                        a1T_dram[half * P : (half + 1) * P,
                                 w * c.WD : (w + 1) * c.WD],
                        a1c[:, 0 : c.WD])

            edge_phase(h_tab, c.T1, ad1_sl, c.HEADS, c.HEADS, c.CH, c.T1, epi1)

        # ---------------- phase 3: L2 matmul ----------------
        with tc.tile_pool(name="mm2", bufs=1) as mm2, \
             tc.tile_pool(name="mm2w", bufs=2) as mm2w, \
             tc.tile_pool(name="mm2p", bufs=2, space="PSUM") as mm2p:
            a1ts = []
            for k in range(c.KC2):
                t = mm2.tile([P, c.DPCP], bf16, tag=f"a1ts{k}")
                nc.sync.dma_start(t[:], a1T_dram[k * P : (k + 1) * P, :])
                a1ts.append(t)
            w2s = []
            for k in range(c.KC2):
                t = mm2.tile([P, c.T2], bf16, tag=f"w2s{k}")
                nc.sync.dma_start(t[:], W2e_d[k * P : (k + 1) * P, :])
                w2s.append(t)
            for r in range(c.RT):
                ps = mm2p.tile([P, c.T2], f32, space="PSUM", tag="mm2ps")
                for k in range(c.KC2):
                    nc.tensor.matmul(
                        ps[:], lhsT=a1ts[k][:, r * P : (r + 1) * P],
                        rhs=w2s[k][:], start=(k == 0), stop=(k == c.KC2 - 1))
                hsb = mm2w.tile([P, c.T2], bf16, tag="h2sb")
                nc.scalar.copy(hsb[:], ps[:])
                nc.sync.dma_start(h2_bounce[r * P : (r + 1) * P, :], hsb[:])
                nc.sync.dma_start(
                    ad2_sl[r * P : (r + 1) * P, :],
                    hsb[:, c.NCLS + 1 : c.NCLS + 2])

        nc.gpsimd.collective_compute(
            "AllGather", OP.bypass,
            replica_groups=[list(range(c.NCORES))],
            ins=[h2_bounce.ap()[0 : c.DPC, :]],
            outs=[h2_tab.ap()])

        # ---------------- phase 4: L2 edge windows ----------------
        NCH1 = max(LC[w] + HC[w] for w in range(c.NW))
        with tc.tile_pool(name="eg2", bufs=2) as eg, \
             tc.tile_pool(name="emeta2", bufs=1) as emeta, \
             tc.tile_pool(name="ew2", bufs=2) as ew, \
             tc.tile_pool(name="eS2", bufs=NCH1 + 4) as eSp, \
             tc.tile_pool(name="est2", bufs=3) as estp, \
             tc.tile_pool(name="ep12", bufs=2, space="PSUM") as ep1, \
             tc.tile_pool(name="ep22", bufs=2, space="PSUM") as ep2, \
             tc.tile_pool(name="ep32", bufs=2, space="PSUM") as ep3:
            idx_t = emeta.tile([P, max(1, meta["idx_cols"])], mybir.dt.int16)
            nc.sync.dma_start(idx_t[:], idx_d[:])
            dstc_t = emeta.tile([P, meta["TOTC"]], f32)
            nc.sync.dma_start(dstc_t[:], dstc_d[:])

            def edge_phase2():
                gelem = c.T2
                hc = c.NCLS
                for g in groups:
                    GC = g["lo_n"] + g["hi_n"]
                    gt = eg.tile([P, GC, gelem], bf16, tag="gt2")
                    col0 = g["idx_col0"]
                    qn = [0]
                    def gather(sec_n, col_off, out_off, in_ap):
                        if sec_n == 0:
                            return
                        nidx = sec_n * P
                        nc.gpsimd.dma_gather(
                            gt[:, out_off : out_off + sec_n, :], in_ap,
                            idx_t[:, col_off : col_off + nidx // 16],
                            nidx, nidx, gelem, single_packet=False,
                            queue_num=qn[0] % 4)
                        qn[0] += 1
                    gather(g["lo_n"], col0, 0, h2_tab.ap())
                    gather(g["hi_n"], col0 + g["lo_n"] * (P // 16), g["lo_n"],
                           h2_tab.ap()[c.SPLIT :, :])
                    for w in g["ws"]:
                        spans = [g["lo"][w], g["hi"][w]]
                        spans = [(s - g["chunk0"], n) for (s, n) in spans if n]
                        nch = sum(n for _, n in spans)
                        if nch == 0:
                            continue
                        ad_t = ew.tile([P, 1], bf16, tag="ad2")
                        nc.vector.memset(ad_t[:], 0.0)
                        nc.sync.dma_start(
                            ad_t[0 : c.WD, :],
                            ad2_sl[w * c.WD : (w + 1) * c.WD, :])
                        S_tiles = []
                        ed_ps = ep1.tile([P, nch, 1], f32, space="PSUM",
                                         tag="edps2")
                        ci = 0
                        for s0, n in spans:
                            for k in range(n):
                                gci = g["chunk0"] + s0 + k
                                lci = s0 + k
                                dcol = dstc_t[:, gci : gci + 1]
                                S = eSp.tile([P, P], bf16, tag="S2")
                                nc.vector.tensor_scalar(
                                    out=S[:], in0=iro_t[:], scalar1=dcol,
                                    scalar2=None, op0=OP.is_equal)
                                tp = ep3.tile([P, P], f32, space="PSUM",
                                              tag="tp2")
                                nc.tensor.transpose(
                                    out=tp[:], in_=dcol.to_broadcast([P, P]),
                                    identity=ident[:])
                                ST = estp.tile([P, P], bf16, tag="ST2")
                                nc.vector.tensor_tensor(
                                    out=ST[:], in0=tp[:],
                                    in1=ico_t[:].to_broadcast([P, P]),
                                    op=OP.is_equal)
                                nc.tensor.matmul(
                                    ed_ps[:, ci, :], lhsT=ST[:], rhs=ad_t[:],
                                    start=True, stop=True)
                                S_tiles.append((S, lci))
                                ci += 1
                        e_t = ew.tile([P, nch, 1], f32, tag="e2")
                        ci = 0
                        for s0, n in spans:
                            nc.vector.tensor_tensor(
                                out=e_t[:, ci : ci + n, :],
                                in0=gt[:, s0 : s0 + n, hc : hc + 1],
                                in1=ed_ps[:, ci : ci + n, :], op=OP.add)
                            ci += n
                        lr_t = ew.tile([P, nch, 1], f32, tag="lr2")
                        nc.vector.tensor_scalar_mul(lr_t[:], e_t[:], c.NEG)
                        nc.vector.tensor_tensor(
                            out=lr_t[:], in0=lr_t[:], in1=e_t[:], op=OP.max)
                        w_t = ew.tile([P, nch, 1], bf16, tag="w2")
                        nc.scalar.activation(w_t[:], lr_t[:], AT.Exp)
                        s_ps = ep2.tile([P, 1], f32, space="PSUM", tag="sps2")
                        for ci, (S, lci) in enumerate(S_tiles):
                            nc.tensor.matmul(
                                s_ps[:], lhsT=S[:], rhs=w_t[:, ci, :],
                                start=(ci == 0), stop=(ci == len(S_tiles) - 1))
                        ci = 0
                        for s0, n in spans:
                            nc.vector.tensor_tensor(
                                out=gt[:, s0 : s0 + n, 0 : hc].rearrange(
                                    "p c (h x) -> p c h x", h=1),
                                in0=gt[:, s0 : s0 + n, 0 : hc].rearrange(
                                    "p c (h x) -> p c h x", h=1),
                                in1=w_t[:, ci : ci + n, :].to_broadcast(
                                    [P, n, 1, hc]),
                                op=OP.mult)
                            ci += n
                        out_ps = ep2.tile([P, hc], f32, space="PSUM",
                                          tag="ops2")
                        k = 0
                        for s0, n in spans:
                            for j in range(n):
                                S, lci = S_tiles[k]; k += 1
                                nc.tensor.matmul(
                                    out_ps[:], lhsT=S[:],
                                    rhs=gt[:, lci, 0 : hc],
                                    start=(k == 1), stop=(k == len(S_tiles)))
                        # epilogue
                        s_sb = ew.tile([P, 1], f32, tag="ssb2")
                        nc.vector.tensor_scalar_add(s_sb[:], s_ps[:], 1e-16)
                        rs = ew.tile([P, 1], f32, tag="rs2")
                        nc.vector.reciprocal(rs[:], s_sb[:])
                        z = ew.tile([P, c.NCLS], f32, tag="z2")
                        nc.vector.tensor_tensor(
                            out=z[:], in0=out_ps[:],
                            in1=rs[:].to_broadcast([P, c.NCLS]), op=OP.mult)
                        nc.vector.tensor_add(out=z[:], in0=z[:], in1=b2r_t[:])
                        nc.sync.dma_start(
                            out_d[w * c.WD : (w + 1) * c.WD, :],
                            z[0 : c.WD, :])

            edge_phase2()

    nc.compile()
    return nc


_CACHE = {}


def kernel(**inputs):
    from concourse.bass_utils import run_bass_kernel_spmd

    cfg = Cfg()
    x = np.asarray(inputs["x"], np.float32)
    ei = np.asarray(inputs["edge_index"], np.int64)
    meta, in_maps = preprocess(
        cfg, x, ei, inputs["W1"], inputs["att_src1"], inputs["att_dst1"],
        inputs["b1"], inputs["W2"], inputs["att_src2"], inputs["att_dst2"],
        inputs["b2"])
    key = (meta["TOTC"], meta["idx_cols"], tuple(meta["LC"]), tuple(meta["HC"]))
    if key not in _CACHE:
        _CACHE[key] = build_program(cfg, meta)
    nc = _CACHE[key]
    res = run_bass_kernel_spmd(nc, in_maps, core_ids=list(range(cfg.NCORES)))
    out = np.concatenate([res.results[co]["out"] for co in range(cfg.NCORES)],
                         axis=0)
    return out.astype(np.float32)


# revision 6
# speedup vs baseline: 1.5475x; 1.5475x over previous
"""GAT (2-layer, 8-head) Trainium2 Bass kernel, 8-core SPMD.

Strategy (dst-sharded edge partition):
- Host: append self-loops, shard edges by dst range (6250 dsts/core), bucket
  into 50 windows of 125 dsts, split each window's edges by src<32768 (lo/hi
  for int16 dma_gather indexing), pad sections to 128-edge chunks with
  SPMD-uniform (max-over-cores) static sizes. One-hot chunk selection
  matrices S (edge->dstcol) and S^T are precomputed host-side as fp8 (0/1).
- Device phase 1: sharded matmul x_sliceT @ W1ext -> h rows
  [h(256) | a_src(8) | a_dst(8) | pad] bf16, AllGather -> full 50000-row table.
- Device phase 2 (per window): dma_gather h[src] rows (768B); e =
  lrelu(a_src[src] + a_dst[dst]) with a_dst broadcast per edge via the S^T
  matmul; w = exp(e) written into the gathered tile's a_src columns; one
  fused matmul per chunk accumulates both the weighted aggregation and the
  per-dst softmax denominators in PSUM; out = agg/s; +b1, ELU -> act1
  (stored transposed for the next matmul).
- Phase 3: act1T @ W2ext -> h2 rows [h2(10)|a_s2|a_d2|pad] bf16, AllGather.
- Phase 4: same edge pipeline with 1 head, 10 channels -> final [6250, 10]
  fp32 slice per core; host concatenates.
"""
import os
import sys
from contextlib import ExitStack

for _p in ("/opt/trn_rl_repo", os.path.expanduser("~/.axon_site/_ro/trn_rl_repo")):
    if os.path.isdir(_p) and _p not in sys.path:
        sys.path.insert(0, _p)

import numpy as np
import ml_dtypes

P = 128


class Cfg:
    def __init__(self, N=50000, F=767, HEADS=8, CH=32, NCLS=10, NCORES=8,
                 WD=125, NW=50, SPLIT=32768, G=2, NEG=0.2):
        self.N, self.F, self.HEADS, self.CH, self.NCLS = N, F, HEADS, CH, NCLS
        self.NCORES, self.WD, self.NW, self.SPLIT, self.G, self.NEG = (
            NCORES, WD, NW, SPLIT, G, NEG)
        self.HID = HEADS * CH                      # 256
        self.DPC = WD * NW                         # dsts per core
        assert self.DPC * NCORES == N
        self.FP = (F + P - 1) // P * P             # padded F
        self.KC1 = self.FP // P                    # k-chunks layer 1
        self.RT = (self.DPC + P - 1) // P          # row tiles per core
        self.DPCP = self.RT * P                    # padded rows/core
        self.W1C = self.HID + 2 * HEADS            # 272 used cols
        self.T1 = 384                              # padded L1 table width (768B rows)
        assert self.T1 * 2 % 256 == 0 and self.W1C <= self.T1
        self.KC2 = self.HID // P                   # 2
        self.W2C = NCLS + 2                        # 12 used cols
        self.T2 = 128                              # padded L2 table width (256B rows)


def _wrap_idxs(idx_list):
    """int16 idx list -> [128, ceil(n/16)] wrapped (p=j%16, col=j//16), x8."""
    n = len(idx_list)
    cols = max(1, (n + 15) // 16)
    arr = np.zeros((16, cols), dtype=np.int16)
    if n:
        j = np.arange(n)
        arr[j % 16, j // 16] = idx_list
    return np.tile(arr, (8, 1))


def preprocess(cfg, x, edge_index, W1, att_src1, att_dst1, b1, W2, att_src2,
               att_dst2, b2):
    c = cfg
    N = c.N
    src = np.concatenate([edge_index[0], np.arange(N)]).astype(np.int64)
    dst = np.concatenate([edge_index[1], np.arange(N)]).astype(np.int64)

    # --- weight prep (param folding only) ---
    W1 = np.asarray(W1, np.float32)
    a_s1 = np.asarray(att_src1, np.float32)
    a_d1 = np.asarray(att_dst1, np.float32)
    W1e = np.zeros((c.FP, c.T1), np.float32)
    W1e[: c.F, : c.HID] = W1
    for h in range(c.HEADS):
        blk = W1[:, h * c.CH : (h + 1) * c.CH]
        W1e[: c.F, c.HID + h] = blk @ a_s1[h]
        W1e[: c.F, c.HID + c.HEADS + h] = blk @ a_d1[h]
    W2 = np.asarray(W2, np.float32)
    W2e = np.zeros((c.HID, c.T2), np.float32)
    W2e[:, : c.NCLS] = W2
    W2e[:, c.NCLS] = W2 @ np.asarray(att_src2, np.float32)[0]
    W2e[:, c.NCLS + 1] = W2 @ np.asarray(att_dst2, np.float32)[0]

    # --- per-core edge bucketing ---
    core = dst // c.DPC
    dloc = dst - core * c.DPC
    win = dloc // c.WD
    dcol = dloc % c.WD
    lists = [[([], []) for _ in range(c.NW)] for _ in range(c.NCORES)]
    is_lo = src < c.SPLIT
    order = np.lexsort((win, core))
    for i in order:
        co, w = int(core[i]), int(win[i])
        lists[co][w][0 if is_lo[i] else 1].append((int(src[i]), int(dcol[i])))

    def nchunks(n):
        return (n + P - 1) // P

    LC = [max(nchunks(len(lists[co][w][0])) for co in range(c.NCORES))
          for w in range(c.NW)]
    HC = [max(nchunks(len(lists[co][w][1])) for co in range(c.NCORES))
          for w in range(c.NW)]

    # group layout: for each group g of G windows: lo sections then hi sections
    NG = (c.NW + c.G - 1) // c.G
    meta = {"LC": LC, "HC": HC, "NG": NG, "groups": []}
    chunk_cursor = 0
    idx_cols = 0
    for g in range(NG):
        ws = list(range(g * c.G, min((g + 1) * c.G, c.NW)))
        lo_n = sum(LC[w] for w in ws)
        hi_n = sum(HC[w] for w in ws)
        ginfo = {"ws": ws, "lo_n": lo_n, "hi_n": hi_n,
                 "chunk0": chunk_cursor, "idx_col0": idx_cols,
                 "lo": {}, "hi": {}}
        off = chunk_cursor
        for w in ws:
            ginfo["lo"][w] = (off, LC[w]); off += LC[w]
        for w in ws:
            ginfo["hi"][w] = (off, HC[w]); off += HC[w]
        chunk_cursor = off
        idx_cols += (lo_n + hi_n) * (P // 16)
        meta["groups"].append(ginfo)
    TOTC = chunk_cursor
    meta["TOTC"] = TOTC
    meta["idx_cols"] = idx_cols

    # --- per-core arrays ---
    in_maps = []
    xf = np.asarray(x, np.float32)
    for co in range(c.NCORES):
        idx_parts = []
        dstc = np.full((P, TOTC), 126, np.int32)   # slot -> dst col (126=pad)
        for g in meta["groups"]:
            for kind in ("lo", "hi"):
                sec = []
                for w in g["ws"]:
                    pairs = lists[co][w][0 if kind == "lo" else 1]
                    cstart, ccount = g[kind][w]
                    nslots = ccount * P
                    vals = np.zeros(nslots, np.int16)
                    for j, (s, dc) in enumerate(pairs):
                        vals[j] = s if kind == "lo" else s - c.SPLIT
                        dstc[j % P, cstart + j // P] = dc
                    sec.append(vals)
                sec = np.concatenate(sec) if sec else np.zeros(0, np.int16)
                idx_parts.append(_wrap_idxs(sec) if len(sec) else
                                 np.zeros((P, 1), np.int16)[:, :0])
        idx_np = (np.concatenate(idx_parts, axis=1) if idx_parts
                  else np.zeros((P, 0), np.int16))
        assert idx_np.shape[1] == meta["idx_cols"]

        # one-hot S [slot_p, chunk, dstcol] and ST [dstcol_p, chunk, slot], fp8
        ar = np.arange(P)
        S_host = (dstc[:, :, None] == ar[None, None, :])
        ST_host = (dstc.T[None, :, :] == ar[:, None, None])  # [j, chunk, e]
        S_host = S_host.astype(ml_dtypes.float8_e4m3).reshape(P, TOTC * P)
        ST_host = ST_host.astype(ml_dtypes.float8_e4m3).reshape(P, TOTC * P)

        xT = np.zeros((c.FP, c.DPCP), ml_dtypes.bfloat16)
        xs = xf[co * c.DPC : (co + 1) * c.DPC]
        xT[: c.F, : c.DPC] = xs.T.astype(ml_dtypes.bfloat16)

        in_maps.append({
            "xT": xT,
            "W1e": W1e.astype(ml_dtypes.bfloat16),
            "W2e": W2e.astype(ml_dtypes.bfloat16),
            "idx": idx_np,
            "Sh": S_host,
            "STh": ST_host,
            "b1r": np.tile(np.asarray(b1, np.float32)[None, :], (P, 1)),
            "b2r": np.tile(np.asarray(b2, np.float32)[None, :], (P, 1)),
        })
    return meta, in_maps


def build_program(cfg, meta):
    import concourse.bacc as bacc
    import concourse.bass as bass
    import concourse.mybir as mybir
    import concourse.tile as tile
    from concourse.library_config import mlp
    from concourse.masks import make_identity

    c = cfg
    f32, bf16 = mybir.dt.float32, mybir.dt.bfloat16
    fp8 = mybir.dt.float8e4
    AT = mybir.ActivationFunctionType
    OP = mybir.AluOpType

    nc = bacc.Bacc("TRN2", target_bir_lowering=False, debug=False,
                   num_devices=c.NCORES, num_swdge_queues=4)
    TOTC = meta["TOTC"]
    xT_d = nc.dram_tensor("xT", [c.FP, c.DPCP], bf16, kind="ExternalInput")
    W1e_d = nc.dram_tensor("W1e", [c.FP, c.T1], bf16, kind="ExternalInput")
    W2e_d = nc.dram_tensor("W2e", [c.HID, c.T2], bf16, kind="ExternalInput")
    idx_d = nc.dram_tensor("idx", [P, max(1, meta["idx_cols"])], mybir.dt.int16,
                           kind="ExternalInput")
    Sh_d = nc.dram_tensor("Sh", [P, TOTC * P], fp8, kind="ExternalInput")
    STh_d = nc.dram_tensor("STh", [P, TOTC * P], fp8, kind="ExternalInput")
    b1r_d = nc.dram_tensor("b1r", [P, c.HID], f32, kind="ExternalInput")
    b2r_d = nc.dram_tensor("b2r", [P, c.NCLS], f32, kind="ExternalInput")
    out_d = nc.dram_tensor("out", [c.DPC, c.NCLS], f32, kind="ExternalOutput")

    _shared = "Shared" if c.NCORES > 4 else "Local"
    h_bounce = nc.dram_tensor("h_bounce", [c.DPCP, c.T1], bf16, kind="Internal")
    h_tab = nc.dram_tensor("h_tab", [c.N, c.T1], bf16, kind="Internal",
                           addr_space=_shared)
    ad1_sl = nc.dram_tensor("ad1_sl", [c.DPCP, c.HEADS], bf16, kind="Internal")
    a1T_dram = nc.dram_tensor("a1T", [c.HID, c.DPCP], bf16, kind="Internal")
    h2_bounce = nc.dram_tensor("h2_bounce", [c.DPCP, c.T2], bf16, kind="Internal")
    h2_tab = nc.dram_tensor("h2_tab", [c.N, c.T2], bf16, kind="Internal",
                            addr_space=_shared)
    ad2_sl = nc.dram_tensor("ad2_sl", [c.DPCP, 1], bf16, kind="Internal")

    groups = meta["groups"]
    LC, HC = meta["LC"], meta["HC"]

    with ExitStack() as stack:
        tc = stack.enter_context(tile.TileContext(nc))
        cpool = stack.enter_context(tc.tile_pool(name="consts", bufs=1))
        nc.gpsimd.load_library(mlp)

        ident = cpool.tile([P, P], f32)
        make_identity(nc, ident[:])
        b1r_t = cpool.tile([P, c.HID], f32)
        nc.sync.dma_start(b1r_t[:], b1r_d[:])
        b2r_t = cpool.tile([P, c.NCLS], f32)
        nc.sync.dma_start(b2r_t[:], b2r_d[:])

        # ---------------- phase 1: L1 matmul (sharded rows) ----------------
        with tc.tile_pool(name="mm1", bufs=1) as mm1, \
             tc.tile_pool(name="mm1w", bufs=3) as mm1w, \
             tc.tile_pool(name="mm1p", bufs=2, space="PSUM") as mm1p:
            xts = []
            for k in range(c.KC1):
                t = mm1.tile([P, c.DPCP], bf16, tag=f"xts{k}")
                nc.sync.dma_start(t[:], xT_d[k * P : (k + 1) * P, :])
                xts.append(t)
            w1s = []
            for k in range(c.KC1):
                t = mm1.tile([P, c.T1], bf16, tag=f"w1s{k}")
                nc.sync.dma_start(t[:], W1e_d[k * P : (k + 1) * P, :])
                w1s.append(t)
            ad_acc = mm1.tile([P, c.RT, c.HEADS], bf16, tag="adacc")
            for r in range(c.RT):
                ps = mm1p.tile([P, c.T1], f32, space="PSUM", tag="mmps")
                for k in range(c.KC1):
                    nc.tensor.matmul(
                        ps[:], lhsT=xts[k][:, r * P : (r + 1) * P],
                        rhs=w1s[k][:], start=(k == 0), stop=(k == c.KC1 - 1))
                hsb = mm1w.tile([P, c.T1], bf16, tag="hsb")
                nc.scalar.copy(hsb[:], ps[:])
                nc.vector.tensor_copy(
                    out=ad_acc[:, r, :],
                    in_=hsb[:, c.HID + c.HEADS : c.HID + 2 * c.HEADS])
                nc.sync.dma_start(h_bounce[r * P : (r + 1) * P, :], hsb[:])
            nc.sync.dma_start(
                ad1_sl.ap().rearrange("(r p) h -> p r h", p=P), ad_acc[:])

        nc.gpsimd.collective_compute(
            "AllGather", OP.bypass,
            replica_groups=[list(range(c.NCORES))],
            ins=[h_bounce.ap()[0 : c.DPC, :]],
            outs=[h_tab.ap()])

        # ---------------- shared edge-window pipeline ----------------
        def edge_phase(pools, tab_d, ad_sl_d, adw, heads, ch, gelem, wcol,
                       rhsw, epilogue):
            """adw: a_dst cols; gelem: gathered row width; wcol: col where
            exp(e) is written in the gather tile; rhsw: agg matmul rhs width
            (ch*heads + adw); epilogue(w, out_ps)."""
            eg, ew, eS, ep1, ep2, ep3 = pools
            hc = heads * ch
            for g in groups:
                GC = g["lo_n"] + g["hi_n"]
                c0 = g["chunk0"]
                gt = eg.tile([P, GC, gelem], bf16, tag="gt")
                S_g = eS.tile([P, GC, P], fp8, tag="Sg")
                nc.sync.dma_start(S_g[:], Sh_d[:, c0 * P : (c0 + GC) * P])
                ST_g = eS.tile([P, GC, P], fp8, tag="STg")
                nc.sync.dma_start(ST_g[:], STh_d[:, c0 * P : (c0 + GC) * P])
                col0 = g["idx_col0"]
                qn = [0]

                def gather(sec_n, col_off, out_off, in_ap):
                    if sec_n == 0:
                        return
                    nidx = sec_n * P
                    nc.gpsimd.dma_gather(
                        gt[:, out_off : out_off + sec_n, :], in_ap,
                        idx_t[:, col_off : col_off + nidx // 16],
                        nidx, nidx, gelem, single_packet=False,
                        queue_num=qn[0] % 4)
                    qn[0] += 1

                gather(g["lo_n"], col0, 0, tab_d.ap())
                gather(g["hi_n"], col0 + g["lo_n"] * (P // 16), g["lo_n"],
                       tab_d.ap()[c.SPLIT :, :])

                for w in g["ws"]:
                    spans = [g["lo"][w], g["hi"][w]]
                    spans = [(s - c0, n) for (s, n) in spans if n]
                    nch = sum(n for _, n in spans)
                    if nch == 0:
                        continue
                    ad_t = ew.tile([P, adw], bf16, tag="ad")
                    nc.vector.memset(ad_t[:], 0.0)
                    nc.sync.dma_start(
                        ad_t[0 : c.WD, :],
                        ad_sl_d[w * c.WD : (w + 1) * c.WD, :])
                    # ed per chunk via S^T matmul
                    ed_ps = ep1.tile([P, nch, adw], f32, space="PSUM",
                                     tag="edps")
                    ci = 0
                    for s0, n in spans:
                        for k in range(n):
                            nc.tensor.matmul(
                                ed_ps[:, ci, :], lhsT=ST_g[:, s0 + k, :],
                                rhs=ad_t[:], start=True, stop=True)
                            ci += 1
                    # e chain, exp written into gt's a_src cols
                    e_t = ew.tile([P, nch, adw], f32, tag="e")
                    ci = 0
                    for s0, n in spans:
                        nc.vector.tensor_tensor(
                            out=e_t[:, ci : ci + n, :],
                            in0=gt[:, s0 : s0 + n, hc : hc + adw],
                            in1=ed_ps[:, ci : ci + n, :], op=OP.add)
                        ci += n
                    lr_t = ew.tile([P, nch, adw], f32, tag="lr")
                    nc.vector.tensor_scalar_mul(lr_t[:], e_t[:], c.NEG)
                    nc.vector.tensor_tensor(
                        out=lr_t[:], in0=lr_t[:], in1=e_t[:], op=OP.max)
                    ci = 0
                    for s0, n in spans:
                        nc.scalar.activation(
                            gt[:, s0 : s0 + n, wcol : wcol + adw],
                            lr_t[:, ci : ci + n, :], AT.Exp)
                        ci += n
                    # msg: h *= w (broadcast over ch)
                    for s0, n in spans:
                        nc.vector.tensor_tensor(
                            out=gt[:, s0 : s0 + n, 0 : hc].rearrange(
                                "p c (h x) -> p c h x", h=heads),
                            in0=gt[:, s0 : s0 + n, 0 : hc].rearrange(
                                "p c (h x) -> p c h x", h=heads),
                            in1=gt[:, s0 : s0 + n, wcol : wcol + adw
                                   ].to_broadcast([P, n, adw, ch]),
                            op=OP.mult)
                    # fused aggregation + denominator matmuls
                    out_ps = ep2.tile([P, rhsw], f32, space="PSUM", tag="ops")
                    k = 0
                    for s0, n in spans:
                        for j in range(n):
                            nc.tensor.matmul(
                                out_ps[:], lhsT=S_g[:, s0 + j, :],
                                rhs=gt[:, s0 + j, 0 : rhsw],
                                start=(k == 0), stop=(k == nch - 1))
                            k += 1
                    epilogue(w, out_ps)

        # ---------------- phase 2: L1 edge windows ----------------
        with tc.tile_pool(name="eg", bufs=2) as eg, \
             tc.tile_pool(name="emeta", bufs=1) as emeta, \
             tc.tile_pool(name="ew", bufs=2) as ew, \
             tc.tile_pool(name="eS", bufs=2) as eS, \
             tc.tile_pool(name="ep1", bufs=2, space="PSUM") as ep1, \
             tc.tile_pool(name="ep2", bufs=2, space="PSUM") as ep2, \
             tc.tile_pool(name="ep3", bufs=2, space="PSUM") as ep3:
            idx_t = emeta.tile([P, max(1, meta["idx_cols"])], mybir.dt.int16)
            nc.sync.dma_start(idx_t[:], idx_d[:])

            def epi1(w, out_ps):
                s_sb = ew.tile([P, c.HEADS], f32, tag="ssb")
                nc.vector.tensor_scalar_add(
                    s_sb[:], out_ps[:, c.HID : c.HID + c.HEADS], 1e-16)
                rs = ew.tile([P, c.HEADS], f32, tag="rs")
                nc.vector.reciprocal(rs[:], s_sb[:])
                z = ew.tile([P, c.HID], f32, tag="z")
                nc.vector.tensor_tensor(
                    out=z[:].rearrange("p (h x) -> p h x", h=c.HEADS),
                    in0=out_ps[:, 0 : c.HID].rearrange(
                        "p (h x) -> p h x", h=c.HEADS),
                    in1=rs[:].to_broadcast([P, c.HEADS, c.CH]), op=OP.mult)
                nc.vector.tensor_add(out=z[:], in0=z[:], in1=b1r_t[:])
                mn = ew.tile([P, c.HID], f32, tag="mn")
                nc.vector.tensor_scalar_min(mn[:], z[:], 0.0)
                em = ew.tile([P, c.HID], f32, tag="em")
                nc.scalar.activation(em[:], mn[:], AT.Exp)
                nc.vector.tensor_scalar_max(mn[:], z[:], 0.0)
                nc.vector.tensor_add(out=em[:], in0=em[:], in1=mn[:])
                nc.vector.tensor_scalar_add(em[:], em[:], -1.0)
                for half in range(c.HID // P):
                    tp = ep3.tile([P, P], f32, space="PSUM", tag="tp")
                    nc.tensor.transpose(
                        out=tp[:], in_=em[:, half * P : (half + 1) * P],
                        identity=ident[:])
                    a1c = ew.tile([P, P], bf16, tag="a1c")
                    nc.scalar.copy(a1c[:], tp[:])
                    nc.sync.dma_start(
                        a1T_dram[half * P : (half + 1) * P,
                                 w * c.WD : (w + 1) * c.WD],
                        a1c[:, 0 : c.WD])

            edge_phase((eg, ew, eS, ep1, ep2, ep3), h_tab, ad1_sl,
                       c.HEADS, c.HEADS, c.CH, c.T1, c.HID, c.HID + c.HEADS,
                       epi1)

        # ---------------- phase 3: L2 matmul ----------------
        with tc.tile_pool(name="mm2", bufs=1) as mm2, \
             tc.tile_pool(name="mm2w", bufs=3) as mm2w, \
             tc.tile_pool(name="mm2p", bufs=2, space="PSUM") as mm2p:
            a1ts = []
            for k in range(c.KC2):
                t = mm2.tile([P, c.DPCP], bf16, tag=f"a1ts{k}")
                nc.sync.dma_start(t[:], a1T_dram[k * P : (k + 1) * P, :])
                a1ts.append(t)
            w2s = []
            for k in range(c.KC2):
                t = mm2.tile([P, c.T2], bf16, tag=f"w2s{k}")
                nc.sync.dma_start(t[:], W2e_d[k * P : (k + 1) * P, :])
                w2s.append(t)
            ad2_acc = mm2.tile([P, c.RT, 1], bf16, tag="ad2acc")
            for r in range(c.RT):
                ps = mm2p.tile([P, c.T2], f32, space="PSUM", tag="mm2ps")
                for k in range(c.KC2):
                    nc.tensor.matmul(
                        ps[:], lhsT=a1ts[k][:, r * P : (r + 1) * P],
                        rhs=w2s[k][:], start=(k == 0), stop=(k == c.KC2 - 1))
                hsb = mm2w.tile([P, c.T2], bf16, tag="h2sb")
                nc.scalar.copy(hsb[:], ps[:])
                nc.vector.tensor_copy(
                    out=ad2_acc[:, r, :],
                    in_=hsb[:, c.NCLS + 1 : c.NCLS + 2])
                nc.sync.dma_start(h2_bounce[r * P : (r + 1) * P, :], hsb[:])
            nc.sync.dma_start(
                ad2_sl.ap().rearrange("(r p) h -> p r h", p=P), ad2_acc[:])

        nc.gpsimd.collective_compute(
            "AllGather", OP.bypass,
            replica_groups=[list(range(c.NCORES))],
            ins=[h2_bounce.ap()[0 : c.DPC, :]],
            outs=[h2_tab.ap()])

        # ---------------- phase 4: L2 edge windows ----------------
        with tc.tile_pool(name="eg2", bufs=2) as eg, \
             tc.tile_pool(name="emeta2", bufs=1) as emeta, \
             tc.tile_pool(name="ew2", bufs=2) as ew, \
             tc.tile_pool(name="eS2", bufs=2) as eS, \
             tc.tile_pool(name="ep12", bufs=2, space="PSUM") as ep1, \
             tc.tile_pool(name="ep22", bufs=2, space="PSUM") as ep2, \
             tc.tile_pool(name="ep32", bufs=2, space="PSUM") as ep3:
            idx_t = emeta.tile([P, max(1, meta["idx_cols"])], mybir.dt.int16)
            nc.sync.dma_start(idx_t[:], idx_d[:])

            def epi2(w, out_ps):
                s_sb = ew.tile([P, 1], f32, tag="ssb2")
                nc.vector.tensor_scalar_add(
                    s_sb[:], out_ps[:, c.NCLS : c.NCLS + 1], 1e-16)
                rs = ew.tile([P, 1], f32, tag="rs2")
                nc.vector.reciprocal(rs[:], s_sb[:])
                z = ew.tile([P, c.NCLS], f32, tag="z2")
                nc.vector.tensor_tensor(
                    out=z[:], in0=out_ps[:, 0 : c.NCLS],
                    in1=rs[:].to_broadcast([P, c.NCLS]), op=OP.mult)
                nc.vector.tensor_add(out=z[:], in0=z[:], in1=b2r_t[:])
                nc.sync.dma_start(
                    out_d[w * c.WD : (w + 1) * c.WD, :], z[0 : c.WD, :])

            edge_phase((eg, ew, eS, ep1, ep2, ep3), h2_tab, ad2_sl,
                       1, 1, c.NCLS, c.T2, c.NCLS, c.NCLS + 1, epi2)

    nc.compile()
    return nc


_CACHE = {}


def kernel(**inputs):
    from concourse.bass_utils import run_bass_kernel_spmd

    cfg = Cfg()
    x = np.asarray(inputs["x"], np.float32)
    ei = np.asarray(inputs["edge_index"], np.int64)
    meta, in_maps = preprocess(
        cfg, x, ei, inputs["W1"], inputs["att_src1"], inputs["att_dst1"],
        inputs["b1"], inputs["W2"], inputs["att_src2"], inputs["att_dst2"],
        inputs["b2"])
    key = (meta["TOTC"], meta["idx_cols"], tuple(meta["LC"]), tuple(meta["HC"]))
    if key not in _CACHE:
        _CACHE[key] = build_program(cfg, meta)
    nc = _CACHE[key]
    res = run_bass_kernel_spmd(nc, in_maps, core_ids=list(range(cfg.NCORES)))
    out = np.concatenate([res.results[co]["out"] for co in range(cfg.NCORES)],
                         axis=0)
    return out.astype(np.float32)


# revision 9
# speedup vs baseline: 1.7121x; 1.1064x over previous
"""GAT (2-layer, 8-head) Trainium2 Bass kernel, 8-core SPMD.

Strategy (dst-sharded edge partition):
- Host: append self-loops, shard edges by dst range (6250 dsts/core), bucket
  into 50 windows of 125 dsts, split each window's edges by src<32768 (lo/hi
  for int16 dma_gather indexing), pad sections to 128-edge chunks with
  SPMD-uniform (max-over-cores) static sizes. One-hot chunk selection
  matrices S (edge->dstcol) and S^T are precomputed host-side as fp8 (0/1).
- Device phase 1: sharded matmul x_sliceT @ W1ext -> h rows
  [h(256) | a_src(8) | a_dst(8) | pad] bf16, AllGather -> full 50000-row table.
- Device phase 2 (per window): dma_gather h[src] rows (768B); e =
  lrelu(a_src[src] + a_dst[dst]) with a_dst broadcast per edge via the S^T
  matmul; w = exp(e) written into the gathered tile's a_src columns; one
  fused matmul per chunk accumulates both the weighted aggregation and the
  per-dst softmax denominators in PSUM; out = agg/s; +b1, ELU -> act1
  (stored transposed for the next matmul).
- Phase 3: act1T @ W2ext -> h2 rows [h2(10)|a_s2|a_d2|pad] bf16, AllGather.
- Phase 4: same edge pipeline with 1 head, 10 channels -> final [6250, 10]
  fp32 slice per core; host concatenates.
"""
import os
import sys
from contextlib import ExitStack

for _p in ("/opt/trn_rl_repo", os.path.expanduser("~/.axon_site/_ro/trn_rl_repo")):
    if os.path.isdir(_p) and _p not in sys.path:
        sys.path.insert(0, _p)

import numpy as np
import ml_dtypes

P = 128


class Cfg:
    def __init__(self, N=50000, F=767, HEADS=8, CH=32, NCLS=10, NCORES=8,
                 WD=125, NW=50, SPLIT=32768, G=2, NEG=0.2):
        self.N, self.F, self.HEADS, self.CH, self.NCLS = N, F, HEADS, CH, NCLS
        self.NCORES, self.WD, self.NW, self.SPLIT, self.G, self.NEG = (
            NCORES, WD, NW, SPLIT, G, NEG)
        self.HID = HEADS * CH                      # 256
        self.DPC = WD * NW                         # dsts per core
        assert self.DPC * NCORES == N
        self.FP = (F + P - 1) // P * P             # padded F
        self.KC1 = self.FP // P                    # k-chunks layer 1
        self.RT = (self.DPC + P - 1) // P          # row tiles per core
        self.DPCP = self.RT * P                    # padded rows/core
        self.W1C = self.HID + 2 * HEADS            # 272 used cols
        self.T1 = 384                              # padded L1 table width (768B rows)
        assert self.T1 * 2 % 256 == 0 and self.W1C <= self.T1
        self.KC2 = self.HID // P                   # 2
        self.W2C = NCLS + 2                        # 12 used cols
        self.T2 = 128                              # padded L2 table width (256B rows)


def _wrap_idxs(idx_list):
    """int16 idx list -> [128, ceil(n/16)] wrapped (p=j%16, col=j//16), x8."""
    n = len(idx_list)
    cols = max(1, (n + 15) // 16)
    arr = np.zeros((16, cols), dtype=np.int16)
    if n:
        j = np.arange(n)
        arr[j % 16, j // 16] = idx_list
    return np.tile(arr, (8, 1))


def preprocess(cfg, x, edge_index, W1, att_src1, att_dst1, b1, W2, att_src2,
               att_dst2, b2):
    c = cfg
    N = c.N
    src = np.concatenate([edge_index[0], np.arange(N)]).astype(np.int64)
    dst = np.concatenate([edge_index[1], np.arange(N)]).astype(np.int64)

    # --- weight prep (param folding only) ---
    W1 = np.asarray(W1, np.float32)
    a_s1 = np.asarray(att_src1, np.float32)
    a_d1 = np.asarray(att_dst1, np.float32)
    W1e = np.zeros((c.FP, c.T1), np.float32)
    W1e[: c.F, : c.HID] = W1
    for h in range(c.HEADS):
        blk = W1[:, h * c.CH : (h + 1) * c.CH]
        W1e[: c.F, c.HID + h] = blk @ a_s1[h]
        W1e[: c.F, c.HID + c.HEADS + h] = blk @ a_d1[h]
    W2 = np.asarray(W2, np.float32)
    W2e = np.zeros((c.HID, c.T2), np.float32)
    W2e[:, : c.NCLS] = W2
    W2e[:, c.NCLS] = W2 @ np.asarray(att_src2, np.float32)[0]
    W2e[:, c.NCLS + 1] = W2 @ np.asarray(att_dst2, np.float32)[0]

    # --- per-core edge bucketing ---
    core = dst // c.DPC
    dloc = dst - core * c.DPC
    win = dloc // c.WD
    dcol = dloc % c.WD
    lists = [[([], []) for _ in range(c.NW)] for _ in range(c.NCORES)]
    is_lo = src < c.SPLIT
    order = np.lexsort((win, core))
    for i in order:
        co, w = int(core[i]), int(win[i])
        lists[co][w][0 if is_lo[i] else 1].append((int(src[i]), int(dcol[i])))

    def nchunks(n):
        return (n + P - 1) // P

    LC = [max(nchunks(len(lists[co][w][0])) for co in range(c.NCORES))
          for w in range(c.NW)]
    HC = [max(nchunks(len(lists[co][w][1])) for co in range(c.NCORES))
          for w in range(c.NW)]

    # group layout: for each group g of G windows: lo sections then hi sections
    NG = (c.NW + c.G - 1) // c.G
    meta = {"LC": LC, "HC": HC, "NG": NG, "groups": []}
    chunk_cursor = 0
    idx_cols = 0
    for g in range(NG):
        ws = list(range(g * c.G, min((g + 1) * c.G, c.NW)))
        lo_n = sum(LC[w] for w in ws)
        hi_n = sum(HC[w] for w in ws)
        ginfo = {"ws": ws, "lo_n": lo_n, "hi_n": hi_n,
                 "chunk0": chunk_cursor, "idx_col0": idx_cols,
                 "lo": {}, "hi": {}}
        off = chunk_cursor
        for w in ws:
            ginfo["lo"][w] = (off, LC[w]); off += LC[w]
        for w in ws:
            ginfo["hi"][w] = (off, HC[w]); off += HC[w]
        chunk_cursor = off
        idx_cols += (lo_n + hi_n) * (P // 16)
        meta["groups"].append(ginfo)
    TOTC = chunk_cursor
    meta["TOTC"] = TOTC
    meta["idx_cols"] = idx_cols
    meta["b1_zero"] = not np.any(np.asarray(b1))
    meta["b2_zero"] = not np.any(np.asarray(b2))

    # --- per-core arrays ---
    in_maps = []
    xf = np.asarray(x, np.float32)
    for co in range(c.NCORES):
        idx_parts = []
        dstc = np.full((P, TOTC), 126, np.int32)   # slot -> dst col (126=pad)
        for g in meta["groups"]:
            for kind in ("lo", "hi"):
                sec = []
                for w in g["ws"]:
                    pairs = lists[co][w][0 if kind == "lo" else 1]
                    cstart, ccount = g[kind][w]
                    nslots = ccount * P
                    vals = np.zeros(nslots, np.int16)
                    for j, (s, dc) in enumerate(pairs):
                        vals[j] = s if kind == "lo" else s - c.SPLIT
                        dstc[j % P, cstart + j // P] = dc
                    sec.append(vals)
                sec = np.concatenate(sec) if sec else np.zeros(0, np.int16)
                idx_parts.append(_wrap_idxs(sec) if len(sec) else
                                 np.zeros((P, 1), np.int16)[:, :0])
        idx_np = (np.concatenate(idx_parts, axis=1) if idx_parts
                  else np.zeros((P, 0), np.int16))
        assert idx_np.shape[1] == meta["idx_cols"]

        # one-hot S [slot_p, chunk, dstcol] and ST [dstcol_p, chunk, slot], fp8
        ar = np.arange(P)
        S_host = (dstc[:, :, None] == ar[None, None, :])
        ST_host = (dstc.T[None, :, :] == ar[:, None, None])  # [j, chunk, e]
        S_host = S_host.astype(ml_dtypes.float8_e4m3).reshape(P, TOTC * P)
        ST_host = ST_host.astype(ml_dtypes.float8_e4m3).reshape(P, TOTC * P)

        xT = np.zeros((c.FP, c.DPCP), ml_dtypes.bfloat16)
        xs = xf[co * c.DPC : (co + 1) * c.DPC]
        xT[: c.F, : c.DPC] = xs.T.astype(ml_dtypes.bfloat16)

        in_maps.append({
            "xT": xT,
            "W1e": W1e.astype(ml_dtypes.bfloat16),
            "W2e": W2e.astype(ml_dtypes.bfloat16),
            "idx": idx_np,
            "Sh": S_host,
            "STh": ST_host,
            "b1r": np.tile(np.asarray(b1, np.float32)[None, :], (P, 1)),
            "b2r": np.tile(np.asarray(b2, np.float32)[None, :], (P, 1)),
        })
    return meta, in_maps


def build_program(cfg, meta):
    import concourse.bacc as bacc
    import concourse.bass as bass
    import concourse.mybir as mybir
    import concourse.tile as tile
    from concourse.library_config import mlp
    from concourse.masks import make_identity

    c = cfg
    f32, bf16 = mybir.dt.float32, mybir.dt.bfloat16
    fp8 = mybir.dt.float8e4
    AT = mybir.ActivationFunctionType
    OP = mybir.AluOpType

    nc = bacc.Bacc("TRN2", target_bir_lowering=False, debug=False,
                   num_devices=c.NCORES, num_swdge_queues=4)
    TOTC = meta["TOTC"]
    xT_d = nc.dram_tensor("xT", [c.FP, c.DPCP], bf16, kind="ExternalInput")
    W1e_d = nc.dram_tensor("W1e", [c.FP, c.T1], bf16, kind="ExternalInput")
    W2e_d = nc.dram_tensor("W2e", [c.HID, c.T2], bf16, kind="ExternalInput")
    idx_d = nc.dram_tensor("idx", [P, max(1, meta["idx_cols"])], mybir.dt.int16,
                           kind="ExternalInput")
    Sh_d = nc.dram_tensor("Sh", [P, TOTC * P], fp8, kind="ExternalInput")
    STh_d = nc.dram_tensor("STh", [P, TOTC * P], fp8, kind="ExternalInput")
    b1r_d = nc.dram_tensor("b1r", [P, c.HID], f32, kind="ExternalInput")
    b2r_d = nc.dram_tensor("b2r", [P, c.NCLS], f32, kind="ExternalInput")
    out_d = nc.dram_tensor("out", [c.DPC, c.NCLS], f32, kind="ExternalOutput")

    _shared = "Shared" if c.NCORES > 4 else "Local"
    h_bounce = nc.dram_tensor("h_bounce", [c.DPCP, c.T1], bf16, kind="Internal")
    h_tab = nc.dram_tensor("h_tab", [c.N, c.T1], bf16, kind="Internal",
                           addr_space=_shared)
    ad1_sl = nc.dram_tensor("ad1_sl", [c.DPCP, c.HEADS], bf16, kind="Internal")
    a1T_dram = nc.dram_tensor("a1T", [c.HID, c.DPCP], bf16, kind="Internal")
    h2_bounce = nc.dram_tensor("h2_bounce", [c.DPCP, c.T2], bf16, kind="Internal")
    h2_tab = nc.dram_tensor("h2_tab", [c.N, c.T2], bf16, kind="Internal",
                            addr_space=_shared)
    ad2_sl = nc.dram_tensor("ad2_sl", [c.DPCP, 1], bf16, kind="Internal")

    groups = meta["groups"]
    LC, HC = meta["LC"], meta["HC"]

    with ExitStack() as stack:
        tc = stack.enter_context(tile.TileContext(nc))
        cpool = stack.enter_context(tc.tile_pool(name="consts", bufs=1))
        nc.gpsimd.load_library(mlp)

        ident = cpool.tile([P, P], f32)
        make_identity(nc, ident[:])
        b1r_t = cpool.tile([P, c.HID], f32)
        nc.sync.dma_start(b1r_t[:], b1r_d[:])
        b2r_t = cpool.tile([P, c.NCLS], f32)
        nc.sync.dma_start(b2r_t[:], b2r_d[:])

        # ---------------- phase 1: L1 matmul (sharded rows) ----------------
        with tc.tile_pool(name="mm1", bufs=1) as mm1, \
             tc.tile_pool(name="mm1w", bufs=3) as mm1w, \
             tc.tile_pool(name="mm1p", bufs=2, space="PSUM") as mm1p:
            xts = []
            for k in range(c.KC1):
                t = mm1.tile([P, c.DPCP], bf16, tag=f"xts{k}")
                nc.sync.dma_start(t[:], xT_d[k * P : (k + 1) * P, :])
                xts.append(t)
            w1s = []
            for k in range(c.KC1):
                t = mm1.tile([P, c.T1], bf16, tag=f"w1s{k}")
                nc.sync.dma_start(t[:], W1e_d[k * P : (k + 1) * P, :])
                w1s.append(t)
            ad_acc = mm1.tile([P, c.RT, c.HEADS], bf16, tag="adacc")
            for r in range(c.RT):
                ps = mm1p.tile([P, c.T1], f32, space="PSUM", tag="mmps")
                for k in range(c.KC1):
                    nc.tensor.matmul(
                        ps[:], lhsT=xts[k][:, r * P : (r + 1) * P],
                        rhs=w1s[k][:], start=(k == 0), stop=(k == c.KC1 - 1))
                hsb = mm1w.tile([P, c.T1], bf16, tag="hsb")
                nc.scalar.copy(hsb[:], ps[:])
                nc.vector.tensor_copy(
                    out=ad_acc[:, r, :],
                    in_=hsb[:, c.HID + c.HEADS : c.HID + 2 * c.HEADS])
                nc.sync.dma_start(h_bounce[r * P : (r + 1) * P, :], hsb[:])
            nc.sync.dma_start(
                ad1_sl.ap().rearrange("(r p) h -> p r h", p=P), ad_acc[:])

        nc.gpsimd.collective_compute(
            "AllGather", OP.bypass,
            replica_groups=[list(range(c.NCORES))],
            ins=[h_bounce.ap()[0 : c.DPC, :]],
            outs=[h_tab.ap()])

        # ---------------- shared edge-window pipeline ----------------
        def edge_phase(pools, tab_d, ad_sl_d, adw, heads, ch, gelem, wcol,
                       rhsw, epilogue):
            """adw: a_dst cols; gelem: gathered row width; wcol: col where
            exp(e) is written in the gather tile; rhsw: agg matmul rhs width
            (ch*heads + adw); epilogue(w, out_ps)."""
            eg, ew, eS, ep1, ep2, ep3 = pools
            hc = heads * ch
            for g in groups:
                GC = g["lo_n"] + g["hi_n"]
                c0 = g["chunk0"]
                gt = eg.tile([P, GC, gelem], bf16, tag="gt")
                S_g = eS.tile([P, GC, P], fp8, tag="Sg")
                nc.sync.dma_start(S_g[:], Sh_d[:, c0 * P : (c0 + GC) * P])
                ST_g = eS.tile([P, GC, P], fp8, tag="STg")
                nc.sync.dma_start(ST_g[:], STh_d[:, c0 * P : (c0 + GC) * P])
                col0 = g["idx_col0"]
                qn = [0]

                def gather(sec_n, col_off, out_off, in_ap):
                    if sec_n == 0:
                        return
                    nidx = sec_n * P
                    nc.gpsimd.dma_gather(
                        gt[:, out_off : out_off + sec_n, :], in_ap,
                        idx_t[:, col_off : col_off + nidx // 16],
                        nidx, nidx, gelem, single_packet=False,
                        queue_num=qn[0] % 4)
                    qn[0] += 1

                gather(g["lo_n"], col0, 0, tab_d.ap())
                gather(g["hi_n"], col0 + g["lo_n"] * (P // 16), g["lo_n"],
                       tab_d.ap()[c.SPLIT :, :])

                # pass A: per-window a_dst load + ed matmuls into one
                # group-level PSUM tile
                ed_ps = ep1.tile([P, GC, adw], f32, space="PSUM", tag="edps")
                win_spans = {}
                for w in g["ws"]:
                    spans = [g["lo"][w], g["hi"][w]]
                    spans = [(s - c0, n) for (s, n) in spans if n]
                    win_spans[w] = spans
                    if not spans:
                        continue
                    ad_t = ew.tile([P, adw], bf16, tag="ad")
                    nc.vector.memset(ad_t[:], 0.0)
                    nc.sync.dma_start(
                        ad_t[0 : c.WD, :],
                        ad_sl_d[w * c.WD : (w + 1) * c.WD, :])
                    for s0, n in spans:
                        for k in range(n):
                            nc.tensor.matmul(
                                ed_ps[:, s0 + k, :], lhsT=ST_g[:, s0 + k, :],
                                rhs=ad_t[:], start=True, stop=True)
                # group-batched e chain: e=a_src+ed, lrelu, exp -> gt w cols
                e_t = ew.tile([P, GC, adw], f32, tag="e")
                nc.vector.tensor_tensor(
                    out=e_t[:], in0=gt[:, :, hc : hc + adw],
                    in1=ed_ps[:], op=OP.add)
                lr_t = ew.tile([P, GC, adw], f32, tag="lr")
                nc.vector.tensor_scalar_mul(lr_t[:], e_t[:], c.NEG)
                nc.vector.tensor_tensor(
                    out=lr_t[:], in0=lr_t[:], in1=e_t[:], op=OP.max)
                nc.scalar.activation(
                    gt[:, :, wcol : wcol + adw], lr_t[:], AT.Exp)
                # group-batched msg: h *= w (broadcast over ch)
                nc.vector.tensor_tensor(
                    out=gt[:, :, 0 : hc].rearrange(
                        "p c (h x) -> p c h x", h=heads),
                    in0=gt[:, :, 0 : hc].rearrange(
                        "p c (h x) -> p c h x", h=heads),
                    in1=gt[:, :, wcol : wcol + adw
                           ].to_broadcast([P, GC, adw, ch]),
                    op=OP.mult)
                # pass B: fused aggregation + denominator matmuls per window
                for w in g["ws"]:
                    spans = win_spans[w]
                    nch = sum(n for _, n in spans)
                    if nch == 0:
                        continue
                    out_ps = ep2.tile([P, rhsw], f32, space="PSUM", tag="ops")
                    k = 0
                    for s0, n in spans:
                        for j in range(n):
                            nc.tensor.matmul(
                                out_ps[:], lhsT=S_g[:, s0 + j, :],
                                rhs=gt[:, s0 + j, 0 : rhsw],
                                start=(k == 0), stop=(k == nch - 1))
                            k += 1
                    epilogue(w, out_ps)

        # ---------------- phase 2: L1 edge windows ----------------
        with tc.tile_pool(name="eg", bufs=2) as eg, \
             tc.tile_pool(name="emeta", bufs=1) as emeta, \
             tc.tile_pool(name="ew", bufs=2) as ew, \
             tc.tile_pool(name="eS", bufs=2) as eS, \
             tc.tile_pool(name="ep1", bufs=2, space="PSUM") as ep1, \
             tc.tile_pool(name="ep2", bufs=2, space="PSUM") as ep2, \
             tc.tile_pool(name="ep3", bufs=2, space="PSUM") as ep3:
            idx_t = emeta.tile([P, max(1, meta["idx_cols"])], mybir.dt.int16)
            nc.sync.dma_start(idx_t[:], idx_d[:])

            def epi1(w, out_ps):
                s_sb = ew.tile([P, c.HEADS], f32, tag="ssb")
                nc.vector.tensor_scalar_add(
                    s_sb[:], out_ps[:, c.HID : c.HID + c.HEADS], 1e-16)
                rs = ew.tile([P, c.HEADS], f32, tag="rs")
                nc.vector.reciprocal(rs[:], s_sb[:])
                z = ew.tile([P, c.HID], f32, tag="z")
                nc.vector.tensor_tensor(
                    out=z[:].rearrange("p (h x) -> p h x", h=c.HEADS),
                    in0=out_ps[:, 0 : c.HID].rearrange(
                        "p (h x) -> p h x", h=c.HEADS),
                    in1=rs[:].to_broadcast([P, c.HEADS, c.CH]), op=OP.mult)
                if not meta.get("b1_zero"):
                    nc.vector.tensor_add(out=z[:], in0=z[:], in1=b1r_t[:])
                # elu(z) = exp(-relu(-z)) + max(z-1, -1)
                r_t = ew.tile([P, c.HID], f32, tag="relu")
                nc.scalar.activation(r_t[:], z[:], AT.Relu, scale=-1.0)
                em = ew.tile([P, c.HID], f32, tag="em")
                nc.scalar.activation(em[:], r_t[:], AT.Exp, scale=-1.0)
                mx = ew.tile([P, c.HID], f32, tag="mx")
                nc.vector.tensor_scalar(
                    out=mx[:], in0=z[:], scalar1=-1.0, scalar2=-1.0,
                    op0=OP.add, op1=OP.max)
                nc.vector.tensor_add(out=em[:], in0=em[:], in1=mx[:])
                for half in range(c.HID // P):
                    tp = ep3.tile([P, P], f32, space="PSUM", tag="tp")
                    nc.tensor.transpose(
                        out=tp[:], in_=em[:, half * P : (half + 1) * P],
                        identity=ident[:])
                    a1c = ew.tile([P, P], bf16, tag="a1c")
                    nc.scalar.copy(a1c[:], tp[:])
                    nc.sync.dma_start(
                        a1T_dram[half * P : (half + 1) * P,
                                 w * c.WD : (w + 1) * c.WD],
                        a1c[:, 0 : c.WD])

            edge_phase((eg, ew, eS, ep1, ep2, ep3), h_tab, ad1_sl,
                       c.HEADS, c.HEADS, c.CH, c.T1, c.HID, c.HID + c.HEADS,
                       epi1)

        # ---------------- phase 3: L2 matmul ----------------
        with tc.tile_pool(name="mm2", bufs=1) as mm2, \
             tc.tile_pool(name="mm2w", bufs=3) as mm2w, \
             tc.tile_pool(name="mm2p", bufs=2, space="PSUM") as mm2p:
            a1ts = []
            for k in range(c.KC2):
                t = mm2.tile([P, c.DPCP], bf16, tag=f"a1ts{k}")
                nc.sync.dma_start(t[:], a1T_dram[k * P : (k + 1) * P, :])
                a1ts.append(t)
            w2s = []
            for k in range(c.KC2):
                t = mm2.tile([P, c.T2], bf16, tag=f"w2s{k}")
                nc.sync.dma_start(t[:], W2e_d[k * P : (k + 1) * P, :])
                w2s.append(t)
            ad2_acc = mm2.tile([P, c.RT, 1], bf16, tag="ad2acc")
            for r in range(c.RT):
                ps = mm2p.tile([P, c.T2], f32, space="PSUM", tag="mm2ps")
                for k in range(c.KC2):
                    nc.tensor.matmul(
                        ps[:], lhsT=a1ts[k][:, r * P : (r + 1) * P],
                        rhs=w2s[k][:], start=(k == 0), stop=(k == c.KC2 - 1))
                hsb = mm2w.tile([P, c.T2], bf16, tag="h2sb")
                nc.scalar.copy(hsb[:], ps[:])
                nc.vector.tensor_copy(
                    out=ad2_acc[:, r, :],
                    in_=hsb[:, c.NCLS + 1 : c.NCLS + 2])
                nc.sync.dma_start(h2_bounce[r * P : (r + 1) * P, :], hsb[:])
            nc.sync.dma_start(
                ad2_sl.ap().rearrange("(r p) h -> p r h", p=P), ad2_acc[:])

        nc.gpsimd.collective_compute(
            "AllGather", OP.bypass,
            replica_groups=[list(range(c.NCORES))],
            ins=[h2_bounce.ap()[0 : c.DPC, :]],
            outs=[h2_tab.ap()])

        # ---------------- phase 4: L2 edge windows ----------------
        with tc.tile_pool(name="eg2", bufs=2) as eg, \
             tc.tile_pool(name="emeta2", bufs=1) as emeta, \
             tc.tile_pool(name="ew2", bufs=2) as ew, \
             tc.tile_pool(name="eS2", bufs=2) as eS, \
             tc.tile_pool(name="ep12", bufs=2, space="PSUM") as ep1, \
             tc.tile_pool(name="ep22", bufs=2, space="PSUM") as ep2, \
             tc.tile_pool(name="ep32", bufs=2, space="PSUM") as ep3:
            idx_t = emeta.tile([P, max(1, meta["idx_cols"])], mybir.dt.int16)
            nc.sync.dma_start(idx_t[:], idx_d[:])

            def epi2(w, out_ps):
                s_sb = ew.tile([P, 1], f32, tag="ssb2")
                nc.vector.tensor_scalar_add(
                    s_sb[:], out_ps[:, c.NCLS : c.NCLS + 1], 1e-16)
                rs = ew.tile([P, 1], f32, tag="rs2")
                nc.vector.reciprocal(rs[:], s_sb[:])
                z = ew.tile([P, c.NCLS], f32, tag="z2")
                nc.vector.tensor_tensor(
                    out=z[:], in0=out_ps[:, 0 : c.NCLS],
                    in1=rs[:].to_broadcast([P, c.NCLS]), op=OP.mult)
                if not meta.get("b2_zero"):
                    nc.vector.tensor_add(out=z[:], in0=z[:], in1=b2r_t[:])
                nc.sync.dma_start(
                    out_d[w * c.WD : (w + 1) * c.WD, :], z[0 : c.WD, :])

            edge_phase((eg, ew, eS, ep1, ep2, ep3), h2_tab, ad2_sl,
                       1, 1, c.NCLS, c.T2, c.NCLS, c.NCLS + 1, epi2)

    nc.compile()
    return nc


_CACHE = {}


def kernel(**inputs):
    from concourse.bass_utils import run_bass_kernel_spmd

    cfg = Cfg()
    x = np.asarray(inputs["x"], np.float32)
    ei = np.asarray(inputs["edge_index"], np.int64)
    meta, in_maps = preprocess(
        cfg, x, ei, inputs["W1"], inputs["att_src1"], inputs["att_dst1"],
        inputs["b1"], inputs["W2"], inputs["att_src2"], inputs["att_dst2"],
        inputs["b2"])
    key = (meta["TOTC"], meta["idx_cols"], tuple(meta["LC"]), tuple(meta["HC"]),
           meta["b1_zero"], meta["b2_zero"])
    if key not in _CACHE:
        _CACHE[key] = build_program(cfg, meta)
    nc = _CACHE[key]
    res = run_bass_kernel_spmd(nc, in_maps, core_ids=list(range(cfg.NCORES)))
    out = np.concatenate([res.results[co]["out"] for co in range(cfg.NCORES)],
                         axis=0)
    return out.astype(np.float32)
